# revision 38
# baseline (speedup 1.0000x reference)
"""Expert-parallel MoE SwiGLU kernel for 8 Trainium2 NeuronCores.

Problem: N=4096 tokens, top-2 of E=8 experts, H=2048, I=1408, fp32.

Strategy (expert parallel, per the sharding hint):
  - Host-side dispatch: gather each expert's routed tokens (the "all-to-all
    dispatch" step) while sharding the full inputs; core e gets expert e's
    token slab x_e^T [H, C] plus its weight triple (transposed).
  - Device: each core independently computes
        y_e^T = Wd_e @ (silu(Wg_e @ x_e^T) * (Wu_e @ x_e^T))
    entirely in [feature, token] layout so no on-device transposes are
    needed. Matmuls run in bf16 (single-pass PE rate, FWL weight loads,
    halved DMA footprint; abs-max rel err vs the fp32 reference ~4e-3).
    Set MOE_KERNEL_CONFIG=f32r for FP22 single-pass compute (~3e-4 rel
    err, ~11% slower end-to-end).
  - Host-side combine: weighted scatter-add of per-expert outputs back to
    the [N, H] output (the "all-to-all combine" step).

All shapes/sharding are hardcoded for this problem instance; capacity C
(max tokens routed to one expert, padded to a multiple of 128) is computed
from the actual routing at call time and the NEFF is compiled per C
(cached within the process).
"""

import numpy as np

import concourse.bass as bass
import concourse.tile as tile
from concourse import bacc, mybir
from concourse import bass_utils

N, K, E, H, I = 4096, 2, 8, 2048, 1408
P = 128
HCH = H // P   # 16 chunks over hidden dim
ICH = I // P   # 11 chunks over intermediate dim
F32 = mybir.dt.float32
F32R = mybir.dt.float32r


def _chunks(C):
    """Split C (multiple of 128) into free-dim chunks, each a multiple of 128
    in [256, 512] (fp32r matmul needs moving dim >= 256 for full rate)."""
    assert C % P == 0 and C >= 2 * P
    k = C // P
    n = (k + 3) // 4  # number of chunks, each <= 512
    if k < 2 * n:     # guarantee every chunk >= 256
        n = k // 2
    base, rem = divmod(k, n)
    sizes = [(base + (1 if i < rem else 0)) * P for i in range(n)]
    out, off = [], 0
    for s in sizes:
        out.append((off, s))
        off += s
    assert off == C and all(s >= 256 and s <= 512 for _, s in out)
    return out


BF16 = mybir.dt.bfloat16


def _build(C, xdt=F32R, wdt=F32R, hdt=F32R):
    """Build + compile the per-core SwiGLU kernel for capacity C."""
    ch = _chunks(C)
    nc = bacc.Bacc("TRN2", target_bir_lowering=False, debug=False,
                   enable_asserts=False, num_devices=E)

    xT = nc.dram_tensor("xT", [H, C], xdt, kind="ExternalInput")
    # weights come host-pre-tiled so every DMA line is contiguous:
    # wgp[i, p, h*128+j] = Wg[e][i*128+j, h*128+p]  (lhsT tiles back to back)
    wgp = nc.dram_tensor("wgp", [ICH, P, H], wdt, kind="ExternalInput")
    wup = nc.dram_tensor("wup", [ICH, P, H], wdt, kind="ExternalInput")
    wdp = nc.dram_tensor("wdp", [HCH, P, I], wdt, kind="ExternalInput")
    outT = nc.dram_tensor("outT", [H, C], F32, kind="ExternalOutput")

    x_r = xT.ap().rearrange("(ho p) c -> p ho c", p=P)      # [128, 16, C]
    wg_r = wgp.ap()
    wu_r = wup.ap()
    wd_r = wdp.ap()
    out_r = outT.ap().rearrange("(ho p) c -> p ho c", p=P)  # [128, 16, C]

    with tile.TileContext(nc) as tc:
        with (
            tc.tile_pool(name="xpool", bufs=1) as xpool,
            tc.tile_pool(name="hpool", bufs=1) as hpool,
            tc.tile_pool(name="wpool", bufs=2) as wpool,
            tc.tile_pool(name="dpool", bufs=2) as dpool,
            tc.tile_pool(name="opool", bufs=2) as opool,
        ):
            # resident activations: x^T and hidden^T
            # Front-load choreography: i=0's gate weights and x h-chunks are
            # interleaved on the SP ring in consumption order so the PE can
            # start at ~8us and track the stream; later weights ride the
            # gpsimd SWDGE ring so they never stall the x stream.
            x_sb = xpool.tile([P, HCH, C], xdt)
            hid_sb = hpool.tile([P, ICH, C], hdt)
            w_sb0 = wpool.tile([P, 2, H], wdt, tag="w12", name="w_sb_0")
            # sync ring: i=0 weights + x h-chunks in consumption order
            from concourse.tile import add_dep_helper
            x_dma = {}
            # x[h0]/x[h1] ride the otherwise-idle ACT HWDGE ring so they
            # land in parallel with the first gate-weight chunks on SP
            x_dma[0] = nc.scalar.dma_start(x_sb[:, 0, :], x_r[:, 0, :])
            x_dma[1] = nc.scalar.dma_start(x_sb[:, 1, :], x_r[:, 1, :])
            nc.sync.dma_start(w_sb0[:, 0, 0:2 * P], wg_r[0][:, 0:2 * P])
            nc.sync.dma_start(w_sb0[:, 0, 2 * P:6 * P], wg_r[0][:, 2 * P:6 * P])
            x_dma[2] = nc.sync.dma_start(x_sb[:, 2, :], x_r[:, 2, :])
            nc.sync.dma_start(w_sb0[:, 0, 6 * P:], wg_r[0][:, 6 * P:])
            for h in range(3, 8):
                x_dma[h] = nc.sync.dma_start(x_sb[:, h, :], x_r[:, h, :])
            nc.sync.dma_start(w_sb0[:, 1, 0:8 * P], wu_r[0][:, 0:8 * P])
            for h in range(8, HCH):
                x_dma[h] = nc.sync.dma_start(x_sb[:, h, :], x_r[:, h, :])
            nc.sync.dma_start(w_sb0[:, 1, 8 * P:], wu_r[0][:, 8 * P:])

            # ---- phase 1: gate/up projections + SwiGLU -> hidden^T [I, C]
            with tc.tile_pool(name="ps1", bufs=1, space="PSUM") as ps1:
                for i in range(ICH):
                    if i == 0:
                        w_sb = w_sb0
                    else:
                        w_sb = wpool.tile([P, 2, H], wdt, tag="w12",
                                          name=f"w_sb_{i}")
                        d0 = nc.gpsimd.dma_start(w_sb[:, 0], wg_r[i])
                        d1 = nc.gpsimd.dma_start(w_sb[:, 1], wu_r[i])
                        if i == 1:
                            # hold the i=1 prefetch until x is nearly done so
                            # the front HBM bandwidth all goes to x (w_i1 is
                            # not consumed until i=0's matmuls finish anyway)
                            add_dep_helper(d0.ins, x_dma[10].ins,
                                           reason="yield front BW to x")
                            add_dep_helper(d1.ins, x_dma[12].ins,
                                           reason="yield front BW to x")
                    ps_g = [
                        ps1.tile([P, cw], F32, name=f"psg_{i}_{n}", tag=f"psg{n}")
                        for n, (c0, cw) in enumerate(ch)
                    ]
                    ps_u = [
                        ps1.tile([P, cw], F32, name=f"psu_{i}_{n}", tag=f"psu{n}")
                        for n, (c0, cw) in enumerate(ch)
                    ]
                    for m, ps in ((0, ps_g), (1, ps_u)):
                        for h in range(HCH):
                            lhsT = w_sb[:, m, h * P:(h + 1) * P]
                            for n, (c0, cw) in enumerate(ch):
                                nc.tensor.matmul(
                                    ps[n][:],
                                    lhsT,
                                    x_sb[:, h, c0:c0 + cw],
                                    start=(h == 0),
                                    stop=(h == HCH - 1),
                                )
                    for n, (c0, cw) in enumerate(ch):
                        hs = hid_sb[:, i, c0:c0 + cw]
                        nc.scalar.activation(
                            out=hs, in_=ps_g[n][:],
                            func=mybir.ActivationFunctionType.Silu,
                        )
                        nc.vector.tensor_mul(out=hs, in0=hs, in1=ps_u[n][:])

            # ---- phase 2: down projection -> out^T [H, C]
            with tc.tile_pool(name="ps2", bufs=2, space="PSUM") as ps2:
                for h in range(HCH):
                    wd_sb = dpool.tile([P, I], wdt, tag="wd")
                    dd = nc.gpsimd.dma_start(wd_sb[:], wd_r[h])
                    if h < 2:
                        # same: keep early down-weight prefetch off the front
                        add_dep_helper(dd.ins, x_dma[15].ins,
                                       reason="yield front BW to x")
                    ps_d = [
                        ps2.tile([P, cw], F32, name=f"psd_{h}_{n}", tag=f"psd{n}")
                        for n, (c0, cw) in enumerate(ch)
                    ]
                    for i in range(ICH):
                        lhsT = wd_sb[:, i * P:(i + 1) * P]
                        for n, (c0, cw) in enumerate(ch):
                            nc.tensor.matmul(
                                ps_d[n][:],
                                lhsT,
                                hid_sb[:, i, c0:c0 + cw],
                                start=(i == 0),
                                stop=(i == ICH - 1),
                            )
                    o_sb = opool.tile([P, C], F32, tag="o")
                    for n, (c0, cw) in enumerate(ch):
                        nc.vector.tensor_copy(o_sb[:, c0:c0 + cw], ps_d[n][:])
                    nc.sync.dma_start(out_r[:, h, :], o_sb[:])

    nc.compile()
    return nc


_NC_CACHE = {}

# compute dtype config: "f32r" (FP22 single-pass, ~3e-4 rel err) or "bf16"
DTYPES = {
    "f32r": (F32R, F32R, F32R),
    "bf16": (BF16, BF16, BF16),
    "xbf16": (BF16, F32R, F32R),
}
import os
CONFIG = os.environ.get("MOE_KERNEL_CONFIG", "bf16")


def _get_nc(C):
    key = (C, CONFIG)
    if key not in _NC_CACHE:
        _NC_CACHE[key] = _build(C, *DTYPES[CONFIG])
    return _NC_CACHE[key]


def kernel(x, topk_ids, topk_weight, Wg, Wu, Wd):
    x = np.asarray(x, dtype=np.float32)
    topk_ids = np.asarray(topk_ids)
    topk_weight = np.asarray(topk_weight, dtype=np.float32)

    # ---- host-side dispatch (the all-to-all by topk_ids)
    flat = topk_ids.reshape(-1).astype(np.int64)
    order = np.argsort(flat, kind="stable")
    counts = np.bincount(flat, minlength=E)
    toks = order // K          # token index per sorted slot
    ks = order % K             # which of the top-k slots
    bounds = np.cumsum(counts)
    starts = bounds - counts

    C = max(2 * P, int(-(-counts.max() // P)) * P)
    nc = _get_nc(C)

    import ml_dtypes
    xdt, wdt, _ = DTYPES[CONFIG]
    np_x = ml_dtypes.bfloat16 if xdt == BF16 else np.float32
    np_w = ml_dtypes.bfloat16 if wdt == BF16 else np.float32

    def pack_gu(w):  # [I, H] -> [ICH, P, H]; out[i, p, h*128+j] = w[i*128+j, h*128+p]
        v = np.asarray(w, np.float32).reshape(ICH, P, HCH, P)       # [i, j, h, p]
        return np.ascontiguousarray(
            v.transpose(0, 3, 2, 1).astype(np_w)).reshape(ICH, P, H)

    def pack_d(w):   # [H, I] -> [HCH, P, I]; out[h, p, i*128+j] = w[h*128+j, i*128+p]
        v = np.asarray(w, np.float32).reshape(HCH, P, ICH, P)       # [h, j, i, p]
        return np.ascontiguousarray(
            v.transpose(0, 3, 2, 1).astype(np_w)).reshape(HCH, P, I)

    in_maps = []
    tok_e, k_e = [], []
    for e in range(E):
        te = toks[starts[e]:bounds[e]]
        ke = ks[starts[e]:bounds[e]]
        tok_e.append(te)
        k_e.append(ke)
        xT_e = np.zeros((H, C), np_x)
        xT_e[:, :len(te)] = x[te].T.astype(np_x)
        in_maps.append({
            "xT": xT_e,
            "wgp": pack_gu(Wg[e]),
            "wup": pack_gu(Wu[e]),
            "wdp": pack_d(Wd[e]),
        })

    res = bass_utils.run_bass_kernel_spmd(nc, in_maps, core_ids=list(range(E)))

    # ---- host-side combine (weighted scatter-add)
    out = np.zeros((N, H), np.float32)
    for e in range(E):
        te, ke = tok_e[e], k_e[e]
        if len(te) == 0:
            continue
        yT = res.results[e]["outT"][:, :len(te)]          # [H, count]
        w = topk_weight[te, ke].astype(np.float32)
        out[te] += (yT * w[None, :]).T
    return out
